# revision 19
# baseline (speedup 1.0000x reference)
"""Trainium2 Bass kernel for nn_CSConv (moe_routing / CFG-routed per-sample conv).

Reference computation (per sample):
  pooled = mean_hw(x)                       # [C_in]
  h      = relu(bn1(pooled @ w_fc1.T))      # [hidden]
  t      = bn2(h @ w_fc2.T + b_fc2)         # [K*C_in]
  theta  = softmax(t / T)                   # [K*C_in] -> [K, C_in]
  w      = einsum('ki,koihw->oihw', theta, anchored_w)
  b      = theta.mean(i) @ anchored_b
  out    = conv2d(x, w, pad=1) + b          # [C_out, H, W]

Sharding: data-parallel over batch, 4 samples per core on 8 cores.

Device algorithm per core (4 samples):
  - x is host-padded to [128, 66, 66] per sample (zero border), conv is done as
    9 accumulating matmuls (one per kernel tap) of [K=128c, M=128o, N=512px].
  - Weight mixing sum_k theta[k,i]*W[k,o,i,t] runs on the tensor engine as
    diag(theta_k) @ aw_k accumulated over k in PSUM (full fp32 accumulate).
  - Matmuls for mix+conv use float32r (full-rate fp32 PE mode, N>=256).
  - Routing (pool/fc/bn/softmax) is fp32 on-device.
"""

import os
import threading

import numpy as np

B, C_IN, H, W = 32, 128, 64, 64
C_OUT, K, KH, KW = 256, 4, 3, 3
HIDDEN = C_IN
T_TEMP = 34.0
BN_EPS = 1e-5
N_CORES = 8
BPC = B // N_CORES            # samples per core
HP, WP = H + 2, W + 2         # padded spatial
NPIX = H * W                  # 4096
PCH = 8                       # pixel chunks per (sample, oc): 8 rows x 64 cols = 512
ROWS_PER_CHUNK = H // PCH     # 8
TAPS = KH * KW                # 9
OC = C_OUT // 128             # 2 output-channel chunks

# conv/mix matmul operand dtype: "f32r" (single-pass fp32, cost model says full
# rate; HW may run it at 1/4) or "bf16" (guaranteed full rate, ~0.3% rel err)
CONV_DTYPE = os.environ.get("CONV_DTYPE", "bf16")

_lock = threading.Lock()
_cache = {}


def _split_multi_waits(nc, mybir):
    """Walrus in this container accepts at most ONE sem wait per instruction.

    Tile emits several (one per dependency lane).  Hoist all but the last wait
    of each instruction onto preceding same-engine NoOps — semantically
    identical (sequencer executes them in program order before the op).
    """
    n = 0
    for fn in nc.m.functions:
        for bb in fn.blocks:
            insts = bb.instructions
            out = []
            changed = False
            for inst in insts:
                si = inst.sync_info
                if si is not None and si.on_wait is not None and len(si.on_wait) > 1:
                    waits = list(si.on_wait)
                    assert inst.engine != mybir.EngineType.Unassigned, inst.name
                    for idx, w in enumerate(waits[:-1]):
                        nop = mybir.InstNoOp(name=f"{inst.name}-xw{idx}", ins=[], outs=[])
                        nop.engine = inst.engine
                        nop.sync_info = mybir.SyncInfo(on_wait=[w], on_update=[])
                        out.append(nop)
                        n += 1
                    si.on_wait = [waits[-1]]
                    inst.sync_info = si
                    changed = True
                if si is not None and si.on_update is not None and len(si.on_update) > 4:
                    raise RuntimeError(f"{inst.name}: {len(si.on_update)} sem updates")
                out.append(inst)
            if changed:
                bb.instructions = out
    return n


def _build_nc():
    import concourse.bass as bass
    import concourse.tile as tile
    from concourse import mybir
    from concourse.masks import make_identity

    f32 = mybir.dt.float32
    f32r = mybir.dt.float32r
    bf16 = mybir.dt.bfloat16
    # cdt: storage dtype of conv/mix matmul operand tiles; rounding/cast copies
    # produce them, matmuls consume them via mm() view.
    if CONV_DTYPE == "bf16":
        cdt = bf16
        mm = lambda ap: ap
    else:
        cdt = f32
        mm = lambda ap: ap.bitcast(f32r)
    AX = mybir.AxisListType
    AF = mybir.ActivationFunctionType

    nc = bass.Bass("TRN2", target_bir_lowering=False, debug=False)

    xp = nc.dram_tensor("xp", [BPC, 128, HP * WP], f32, kind="ExternalInput").ap()
    aw = nc.dram_tensor("aw", [128, K, TAPS * C_OUT], f32, kind="ExternalInput").ap()
    wfc1t = nc.dram_tensor("wfc1t", [128, HIDDEN], f32, kind="ExternalInput").ap()
    wfc2t = nc.dram_tensor("wfc2t", [HIDDEN, K * C_IN], f32, kind="ExternalInput").ap()
    bn1 = nc.dram_tensor("bn1", [128, 4], f32, kind="ExternalInput").ap()  # g,b,m,v cols
    bn2 = nc.dram_tensor("bn2", [4, 5, 512], f32, kind="ExternalInput").ap()  # g,b,m,v,bfc2 (replicated over 4 parts)
    ab = nc.dram_tensor("ab", [K, C_OUT], f32, kind="ExternalInput").ap()
    out = nc.dram_tensor("out", [BPC, C_OUT, NPIX], f32, kind="ExternalOutput").ap()

    out_v = out.rearrange("n (oc p) q -> n oc p q", oc=OC)

    with tile.TileContext(nc) as tc:
        with (
            tc.tile_pool(name="consts", bufs=1) as consts,
            tc.tile_pool(name="xpool", bufs=1) as xpool,
        ):
            # ---- constant loads ----
            wfc1t_sb = consts.tile([128, HIDDEN], f32, name="wfc1t", tag="wfc1t")
            nc.sync.dma_start(wfc1t_sb, wfc1t)
            wfc2t_sb = consts.tile([HIDDEN, K * C_IN], f32, name="wfc2t", tag="wfc2t")
            nc.sync.dma_start(wfc2t_sb, wfc2t)
            bn1_sb = consts.tile([128, 4], f32, name="bn1", tag="bn1")
            nc.sync.dma_start(bn1_sb, bn1)
            bn2_sb = consts.tile([4, 5, 512], f32, name="bn2", tag="bn2")
            nc.sync.dma_start(bn2_sb, bn2)
            ab_sb = consts.tile([K, C_OUT], f32, name="ab", tag="ab")
            nc.sync.dma_start(ab_sb, ab)

            id4 = consts.tile([4, 4], f32, name="id4", tag="id4")
            make_identity(nc, id4)
            id128 = consts.tile([128, 128], f32, name="id128", tag="id128")
            make_identity(nc, id128)

            # ---- loads + f32r rounding copies (walrus requires f32r matmul
            # inputs to be produced by a rounding compute op, not a DMA) ----
            aw_sb = consts.tile([128, K, TAPS * C_OUT], cdt, name="aw", tag="aw")
            xt = []
            pooled = consts.tile([128, BPC], f32, name="pooled", tag="pooled")
            with tc.tile_pool(name="scratch", bufs=2) as scratch:
                aw_raw = scratch.tile(
                    [128, K, TAPS * C_OUT], f32, name="awraw", tag="awraw", bufs=1
                )
                nc.sync.dma_start(aw_raw, aw)
                nc.vector.tensor_copy(mm(aw_sb), aw_raw)
                for b in range(BPC):
                    x_raw = scratch.tile([128, HP, WP], f32, name=f"xraw{b}", tag="xraw")
                    nc.sync.dma_start(x_raw, xp[b].rearrange("p (h w) -> p h w", h=HP))
                    x_b = xpool.tile([128, HP, WP], cdt, name=f"x{b}", tag=f"x{b}")
                    nc.vector.tensor_copy(mm(x_b), x_raw)
                    xt.append(x_b)
                    # zero border contributes nothing to the sum
                    nc.vector.reduce_sum(out=pooled[:, b : b + 1], in_=x_b, axis=AX.XY)
            nc.vector.tensor_scalar_mul(pooled, pooled, 1.0 / NPIX)

            # ---- BN1 fold: s1 = g/sqrt(v+eps), t1 = b - m*s1 ----
            eps128 = consts.tile([128, 1], f32, name="eps128", tag="eps128")
            nc.vector.memset(eps128, BN_EPS)
            eps4 = consts.tile([4, 1], f32, name="eps4", tag="eps4")
            nc.vector.memset(eps4, BN_EPS)
            s1 = consts.tile([128, 1], f32, name="s1", tag="s1")
            t1 = consts.tile([128, 1], f32, name="t1", tag="t1")
            nc.scalar.activation(out=s1, in_=bn1_sb[:, 3:4], func=AF.Sqrt, bias=eps128)
            nc.vector.reciprocal(s1, s1)
            nc.vector.tensor_mul(s1, s1, bn1_sb[:, 0:1])
            nc.vector.tensor_mul(t1, bn1_sb[:, 2:3], s1)
            nc.vector.tensor_sub(t1, bn1_sb[:, 1:2], t1)

            with (
                tc.tile_pool(name="wmixp", bufs=1) as wmixp,
                tc.tile_pool(name="outp", bufs=2) as outp,
                tc.tile_pool(name="ps", bufs=1, space="PSUM") as psc,
            ):
                psr = psm = psc  # one pool; phases share bank tags c0..c7
                # ---- routing ----
                hp_ = psr.tile([128, BPC], f32, name="hp", tag="c0")
                nc.tensor.matmul(hp_, lhsT=wfc1t_sb, rhs=pooled, start=True, stop=True)
                h_sb = consts.tile([128, BPC], f32, name="h", tag="h")
                nc.scalar.activation(out=h_sb, in_=hp_, func=AF.Relu, bias=t1, scale=s1)

                tp = psr.tile([4, K * C_IN], f32, name="tp", tag="c1")
                nc.tensor.matmul(tp, lhsT=h_sb, rhs=wfc2t_sb, start=True, stop=True)

                # a2 = s2/T, c2 = ((bfc2 - m2)*s2 + b2)/T   (free-dim vectors, 4 parts)
                a2 = consts.tile([4, 512], f32, name="a2", tag="a2")
                c2 = consts.tile([4, 512], f32, name="c2", tag="c2")
                nc.scalar.activation(out=a2, in_=bn2_sb[:, 3, :], func=AF.Sqrt, bias=eps4)
                nc.vector.reciprocal(a2, a2)
                nc.vector.tensor_mul(a2, a2, bn2_sb[:, 0, :])          # s2
                nc.vector.tensor_sub(c2, bn2_sb[:, 4, :], bn2_sb[:, 2, :])
                nc.vector.tensor_mul(c2, c2, a2)
                nc.vector.tensor_add(c2, c2, bn2_sb[:, 1, :])
                nc.scalar.mul(c2, c2, 1.0 / T_TEMP)
                nc.scalar.mul(a2, a2, 1.0 / T_TEMP)

                z = consts.tile([4, 512], f32, name="z", tag="z")
                nc.vector.tensor_mul(z, tp, a2)
                nc.vector.tensor_add(z, z, c2)

                # softmax rows
                mx = consts.tile([4, 1], f32, name="mx", tag="mx")
                nc.vector.reduce_max(out=mx, in_=z, axis=AX.X)
                negm = consts.tile([4, 1], f32, name="negm", tag="negm")
                nc.vector.tensor_scalar_mul(negm, mx, -1.0)
                e = consts.tile([4, 512], f32, name="e", tag="e")
                nc.scalar.activation(out=e, in_=z, func=AF.Exp, bias=negm, scale=1.0)
                ssum = consts.tile([4, 1], f32, name="ssum", tag="ssum")
                nc.vector.reduce_sum(out=ssum, in_=e, axis=AX.X)
                rs = consts.tile([4, 1], f32, name="rs", tag="rs")
                nc.vector.reciprocal(rs, ssum)
                theta = consts.tile([4, 512], f32, name="theta", tag="theta")
                nc.vector.tensor_scalar_mul(theta, e, rs)

                # thetamean over i within each k block -> [4(b), K]
                tm = consts.tile([4, K], f32, name="tm", tag="tm")
                thv = theta.rearrange("p (k i) -> p k i", k=K)
                for k in range(K):
                    nc.vector.reduce_sum(out=tm[:, k : k + 1], in_=thv[:, k, :], axis=AX.X)
                nc.vector.tensor_scalar_mul(tm, tm, 1.0 / C_IN)

                tmT_p = psr.tile([K, 4], f32, name="tmTp", tag="c2")
                nc.tensor.transpose(tmT_p, tm, id4)
                tmT = consts.tile([K, 4], f32, name="tmT", tag="tmT")
                nc.vector.tensor_copy(tmT, tmT_p)

                # bias[o, b] = sum_k ab[k, o] * tmT[k, b]
                bias_sb = consts.tile([128, OC, BPC], f32, name="bias", tag="bias")
                for oc in range(OC):
                    bp = psr.tile([128, BPC], f32, name="bp", tag="c3")
                    nc.tensor.matmul(
                        bp, lhsT=ab_sb[:, oc * 128 : (oc + 1) * 128], rhs=tmT,
                        start=True, stop=True,
                    )
                    nc.vector.tensor_copy(bias_sb[:, oc, :], bp)

                # theta transposed to [i, k, b]
                thik = consts.tile([128, K, BPC], f32, name="thik", tag="thik")
                for k in range(K):
                    thp = psr.tile([128, 4], f32, name="thp", tag="c4")
                    nc.tensor.transpose(thp, theta[:, k * 128 : (k + 1) * 128], id4)
                    nc.vector.tensor_copy(thik[:, k, :], thp)

                # Dall[i, b, k, :] = id128 * theta[i, b, k]
                Dall = consts.tile([128, BPC, K, 128], cdt, name="dall", tag="dall")
                for b in range(BPC):
                    for k in range(K):
                        nc.vector.tensor_scalar_mul(
                            mm(Dall[:, b, k, :]), id128, thik[:, k, b : b + 1]
                        )

                # ---- weight mixing on PE: wmix[i, b, :] = sum_k Dall[b,k] @ aw[k] ----
                wmix = wmixp.tile([128, BPC, TAPS * C_OUT], cdt, name="wmix", tag="wmix")
                chunks = [(0, 512), (512, 512), (1024, 512), (1536, 512), (2048, 256)]
                mixi = 0
                for b in range(BPC):
                    for off, ln in chunks:
                        mp = psm.tile([128, 512], f32, name="mix", tag=f"c{mixi % 4}")
                        mixi += 1
                        for k in range(K):
                            nc.tensor.matmul(
                                mp[:, :ln],
                                lhsT=mm(Dall[:, b, k, :]),
                                rhs=mm(aw_sb[:, k, off : off + ln]),
                                start=(k == 0),
                                stop=(k == K - 1),
                            )
                        nc.vector.tensor_copy(
                            mm(wmix[:, b, off : off + ln]), mp[:, :ln]
                        )

                # ---- conv: 9 taps accumulated in psum, tap-outer over all 8
                # pixel-chunk accumulators (8 PSUM banks) for weight reuse ----
                for b in range(BPC):
                    xv = xt[b]
                    for oc in range(OC):
                        cps = [
                            psc.tile([128, 512], f32, name=f"c{pc}", tag=f"c{pc}")
                            for pc in range(PCH)
                        ]
                        for tap in range(TAPS):
                            ta, tb = divmod(tap, KW)
                            woff = tap * C_OUT + oc * 128
                            lhsT = mm(wmix[:, b, woff : woff + 128])
                            for pc in range(PCH):
                                r0 = pc * ROWS_PER_CHUNK
                                rhs = xv[:, r0 + ta : r0 + ta + ROWS_PER_CHUNK, tb : tb + W]
                                nc.tensor.matmul(
                                    cps[pc],
                                    lhsT=lhsT,
                                    rhs=mm(rhs),
                                    start=(tap == 0),
                                    stop=(tap == TAPS - 1),
                                )
                        ot = outp.tile([128, NPIX], f32, name="ot", tag="ot")
                        for pc in range(PCH):
                            nc.scalar.activation(
                                out=ot[:, pc * 512 : (pc + 1) * 512], in_=cps[pc],
                                func=mybir.ActivationFunctionType.Identity,
                                bias=bias_sb[:, oc, b : b + 1], scale=1.0,
                            )
                        dmae = nc.sync if (b * OC + oc) % 2 == 0 else nc.scalar
                        dmae.dma_start(out_v[b, oc], ot)
    return nc


def _get_nc(split=False):
    with _lock:
        if "nc" not in _cache:
            _cache["nc"] = _build_nc()
        if split and not _cache.get("split_done"):
            # CoreSim can't execute the inserted NoOps, so the wait-split
            # post-pass is applied only when targeting hardware.
            from concourse import mybir

            _split_multi_waits(_cache["nc"], mybir)
            _cache["split_done"] = True
        return _cache["nc"]


def _prep_inputs(inputs):
    """Host-side layout prep (pad / transpose / replicate only)."""
    x = np.asarray(inputs["x"], np.float32)
    xp = np.zeros((B, 128, HP, WP), np.float32)
    xp[:, :, 1 : H + 1, 1 : W + 1] = x
    xp = xp.reshape(B, 128, HP * WP)

    # anchored_w [K, O, I, KH, KW] -> [I, K, KH*KW, O] -> [I, K, 9*O]
    awt = np.ascontiguousarray(
        np.asarray(inputs["anchored_w"], np.float32).transpose(2, 0, 3, 4, 1)
    ).reshape(128, K, TAPS * C_OUT)

    wfc1t = np.ascontiguousarray(np.asarray(inputs["w_fc1"], np.float32).T)
    wfc2t = np.ascontiguousarray(np.asarray(inputs["w_fc2"], np.float32).T)

    bn1 = np.stack(
        [
            np.asarray(inputs["bn1_gamma"], np.float32),
            np.asarray(inputs["bn1_beta"], np.float32),
            np.asarray(inputs["bn1_mean"], np.float32),
            np.asarray(inputs["bn1_var"], np.float32),
        ],
        axis=1,
    )  # [128, 4]

    bn2_rows = np.stack(
        [
            np.asarray(inputs["bn2_gamma"], np.float32),
            np.asarray(inputs["bn2_beta"], np.float32),
            np.asarray(inputs["bn2_mean"], np.float32),
            np.asarray(inputs["bn2_var"], np.float32),
            np.asarray(inputs["b_fc2"], np.float32),
        ],
        axis=0,
    )  # [5, 512]
    bn2 = np.ascontiguousarray(np.broadcast_to(bn2_rows[None], (4, 5, 512)))

    ab = np.ascontiguousarray(np.asarray(inputs["anchored_b"], np.float32))

    shared = {
        "aw": awt,
        "wfc1t": wfc1t,
        "wfc2t": wfc2t,
        "bn1": np.ascontiguousarray(bn1),
        "bn2": bn2,
        "ab": ab,
    }
    in_maps = []
    for c in range(N_CORES):
        m = dict(shared)
        m["xp"] = np.ascontiguousarray(xp[c * BPC : (c + 1) * BPC])
        in_maps.append(m)
    return in_maps


def _get_exec():
    """Build (once) a cached jitted shard_map executable over the 8 cores.

    Mirrors concourse.bass2jax.run_bass_via_pjrt but keeps the jitted
    callable across kernel() invocations so repeat calls skip retracing.
    """
    with _lock:
        if "exec" in _cache:
            return _cache["exec"]

    import jax
    from jax.sharding import Mesh, PartitionSpec
    from jax.experimental.shard_map import shard_map
    from concourse import mybir
    from concourse.bass2jax import (
        _bass_exec_p,
        install_neuronx_cc_hook,
        partition_id_tensor,
    )

    nc = _get_nc(split=True)
    install_neuronx_cc_hook()
    assert nc.dbg_addr is None
    partition_name = nc.partition_id_tensor.name if nc.partition_id_tensor else None

    in_names, out_names, out_avals = [], [], []
    for alloc in nc.m.functions[0].allocations:
        if not isinstance(alloc, mybir.MemoryLocationSet):
            continue
        name = alloc.memorylocations[0].name
        if alloc.kind == "ExternalInput":
            if name != partition_name:
                in_names.append(name)
        elif alloc.kind == "ExternalOutput":
            out_names.append(name)
            out_avals.append(
                jax.core.ShapedArray(tuple(alloc.tensor_shape), mybir.dt.np(alloc.dtype))
            )
    n_params = len(in_names)
    n_outs = len(out_avals)
    all_names = list(in_names) + list(out_names)
    if partition_name is not None:
        all_names.append(partition_name)
    all_names = tuple(all_names)
    donate = tuple(range(n_params, n_params + n_outs))

    def _body(*args):
        operands = list(args)
        if partition_name is not None:
            operands.append(partition_id_tensor())
        outs = _bass_exec_p.bind(
            *operands,
            out_avals=tuple(out_avals),
            in_names=all_names,
            out_names=tuple(out_names),
            lowering_input_output_aliases=(),
            sim_require_finite=True,
            sim_require_nnan=True,
            nc=nc,
        )
        return tuple(outs)

    devices = jax.devices()[:N_CORES]
    mesh = Mesh(np.asarray(devices), ("core",))
    in_specs = (PartitionSpec("core"),) * (n_params + n_outs)
    out_specs = (PartitionSpec("core"),) * n_outs
    sharded = jax.jit(
        shard_map(_body, mesh=mesh, in_specs=in_specs, out_specs=out_specs, check_rep=False),
        donate_argnums=donate,
        keep_unused=True,
    )
    sharding = jax.sharding.NamedSharding(mesh, PartitionSpec("core"))
    zero_shapes = [
        ((N_CORES * a.shape[0],) + tuple(a.shape[1:]), a.dtype) for a in out_avals
    ]
    make_zeros = jax.jit(
        lambda: tuple(np.zeros(s, d) * 0 for s, d in zero_shapes),
        out_shardings=(sharding,) * n_outs,
    )
    ex = {
        "sharded": sharded,
        "in_names": in_names,
        "out_names": out_names,
        "out_avals": out_avals,
        "sharding": sharding,
        "zero_shapes": zero_shapes,
        "make_zeros": make_zeros,
    }
    with _lock:
        _cache["exec"] = ex
    return ex


def _concat_inputs(in_maps, in_names):
    return [
        np.concatenate([in_maps[c][name] for c in range(N_CORES)], axis=0)
        for name in in_names
    ]


def kernel(**inputs):
    ex = _get_exec()
    in_maps = _prep_inputs(inputs)
    concat_in = _concat_inputs(in_maps, ex["in_names"])
    zeros = [np.zeros(s, d) for s, d in ex["zero_shapes"]]
    out_arrs = ex["sharded"](*concat_in, *zeros)
    out = np.asarray(out_arrs[0]).reshape(B, C_OUT, H, W)
    return out


def bench(n_iters=30, **inputs):
    """Steady-state per-iteration device time: inputs resident on device,
    outputs kept on device, async dispatch, block at the end."""
    import time

    import jax
    import jax.numpy as jnp

    ex = _get_exec()
    in_maps = _prep_inputs(inputs)
    concat_in = _concat_inputs(in_maps, ex["in_names"])
    dev_in = [jax.device_put(a, ex["sharding"]) for a in concat_in]
    jax.block_until_ready(dev_in)
    sharded = ex["sharded"]
    sharding = ex["sharding"]
    zero_shapes = ex["zero_shapes"]

    def zeros_on_device():
        return [
            jax.device_put(jnp.zeros(s, d), sharding) for s, d in zero_shapes
        ]

    # warmup
    outs = sharded(*dev_in, *zeros_on_device())
    jax.block_until_ready(outs)

    zsets = [zeros_on_device() for _ in range(n_iters)]
    for z in zsets:
        jax.block_until_ready(z)
    t0 = time.perf_counter()
    last = None
    for i in range(n_iters):
        last = sharded(*dev_in, *zsets[i])
    jax.block_until_ready(last)
    dt = (time.perf_counter() - t0) / n_iters
    return dt * 1e9


# revision 20
# speedup vs baseline: 1.0653x; 1.0653x over previous
"""Trainium2 Bass kernel for nn_CSConv (moe_routing / CFG-routed per-sample conv).

Reference computation (per sample):
  pooled = mean_hw(x)                       # [C_in]
  h      = relu(bn1(pooled @ w_fc1.T))      # [hidden]
  t      = bn2(h @ w_fc2.T + b_fc2)         # [K*C_in]
  theta  = softmax(t / T)                   # [K*C_in] -> [K, C_in]
  w      = einsum('ki,koihw->oihw', theta, anchored_w)
  b      = theta.mean(i) @ anchored_b
  out    = conv2d(x, w, pad=1) + b          # [C_out, H, W]

Sharding: data-parallel over batch, 4 samples per core on 8 cores.

Device algorithm per core (4 samples):
  - x is host-padded to [128, 66, 66] per sample (zero border), conv is done as
    9 accumulating matmuls (one per kernel tap) of [K=128c, M=128o, N=512px].
  - Weight mixing sum_k theta[k,i]*W[k,o,i,t] runs on the tensor engine as
    diag(theta_k) @ aw_k accumulated over k in PSUM (full fp32 accumulate).
  - Matmuls for mix+conv use float32r (full-rate fp32 PE mode, N>=256).
  - Routing (pool/fc/bn/softmax) is fp32 on-device.
"""

import os
import threading

import numpy as np

B, C_IN, H, W = 32, 128, 64, 64
C_OUT, K, KH, KW = 256, 4, 3, 3
HIDDEN = C_IN
T_TEMP = 34.0
BN_EPS = 1e-5
N_CORES = 8
BPC = B // N_CORES            # samples per core
HP, WP = H + 2, W + 2         # padded spatial
NPIX = H * W                  # 4096
PCH = 8                       # pixel chunks per (sample, oc): 8 rows x 64 cols = 512
ROWS_PER_CHUNK = H // PCH     # 8
TAPS = KH * KW                # 9
OC = C_OUT // 128             # 2 output-channel chunks

# conv/mix matmul operand dtype: "f32r" (single-pass fp32, cost model says full
# rate; HW may run it at 1/4) or "bf16" (guaranteed full rate, ~0.3% rel err)
CONV_DTYPE = os.environ.get("CONV_DTYPE", "bf16")

_lock = threading.Lock()
_cache = {}


def _split_multi_waits(nc, mybir):
    """Walrus in this container accepts at most ONE sem wait per instruction.

    Tile emits several (one per dependency lane).  Hoist all but the last wait
    of each instruction onto preceding same-engine NoOps — semantically
    identical (sequencer executes them in program order before the op).
    """
    n = 0
    for fn in nc.m.functions:
        for bb in fn.blocks:
            insts = bb.instructions
            out = []
            changed = False
            for inst in insts:
                si = inst.sync_info
                if si is not None and si.on_wait is not None and len(si.on_wait) > 1:
                    waits = list(si.on_wait)
                    assert inst.engine != mybir.EngineType.Unassigned, inst.name
                    for idx, w in enumerate(waits[:-1]):
                        nop = mybir.InstNoOp(name=f"{inst.name}-xw{idx}", ins=[], outs=[])
                        nop.engine = inst.engine
                        nop.sync_info = mybir.SyncInfo(on_wait=[w], on_update=[])
                        out.append(nop)
                        n += 1
                    si.on_wait = [waits[-1]]
                    inst.sync_info = si
                    changed = True
                if si is not None and si.on_update is not None and len(si.on_update) > 4:
                    raise RuntimeError(f"{inst.name}: {len(si.on_update)} sem updates")
                out.append(inst)
            if changed:
                bb.instructions = out
    return n


def _build_nc():
    import concourse.bass as bass
    import concourse.tile as tile
    from concourse import mybir
    from concourse.masks import make_identity

    f32 = mybir.dt.float32
    f32r = mybir.dt.float32r
    bf16 = mybir.dt.bfloat16
    # cdt: storage dtype of conv/mix matmul operand tiles; rounding/cast copies
    # produce them, matmuls consume them via mm() view.
    if CONV_DTYPE == "bf16":
        cdt = bf16
        mm = lambda ap: ap
    else:
        cdt = f32
        mm = lambda ap: ap.bitcast(f32r)
    AX = mybir.AxisListType
    AF = mybir.ActivationFunctionType

    nc = bass.Bass("TRN2", target_bir_lowering=False, debug=False)

    xp = nc.dram_tensor("xp", [BPC, 128, HP * WP], f32, kind="ExternalInput").ap()
    aw = nc.dram_tensor("aw", [128, K, TAPS * C_OUT], f32, kind="ExternalInput").ap()
    wfc1t = nc.dram_tensor("wfc1t", [128, HIDDEN], f32, kind="ExternalInput").ap()
    wfc2t = nc.dram_tensor("wfc2t", [HIDDEN, K * C_IN], f32, kind="ExternalInput").ap()
    bn1 = nc.dram_tensor("bn1", [128, 4], f32, kind="ExternalInput").ap()  # g,b,m,v cols
    bn2 = nc.dram_tensor("bn2", [4, 5, 512], f32, kind="ExternalInput").ap()  # g,b,m,v,bfc2 (replicated over 4 parts)
    ab = nc.dram_tensor("ab", [K, C_OUT], f32, kind="ExternalInput").ap()
    out = nc.dram_tensor("out", [BPC, C_OUT, NPIX], f32, kind="ExternalOutput").ap()

    out_v = out.rearrange("n (oc p) (g q) -> n oc g p q", oc=OC, g=2)

    with tile.TileContext(nc) as tc:
        with (
            tc.tile_pool(name="consts", bufs=1) as consts,
            tc.tile_pool(name="xpool", bufs=1) as xpool,
        ):
            # ---- constant loads ----
            wfc1t_sb = consts.tile([128, HIDDEN], f32, name="wfc1t", tag="wfc1t")
            nc.sync.dma_start(wfc1t_sb, wfc1t)
            wfc2t_sb = consts.tile([HIDDEN, K * C_IN], f32, name="wfc2t", tag="wfc2t")
            nc.sync.dma_start(wfc2t_sb, wfc2t)
            bn1_sb = consts.tile([128, 4], f32, name="bn1", tag="bn1")
            nc.sync.dma_start(bn1_sb, bn1)
            bn2_sb = consts.tile([4, 5, 512], f32, name="bn2", tag="bn2")
            nc.sync.dma_start(bn2_sb, bn2)
            ab_sb = consts.tile([K, C_OUT], f32, name="ab", tag="ab")
            nc.sync.dma_start(ab_sb, ab)

            id4 = consts.tile([4, 4], f32, name="id4", tag="id4")
            make_identity(nc, id4)
            id128 = consts.tile([128, 128], f32, name="id128", tag="id128")
            make_identity(nc, id128)

            # ---- loads + f32r rounding copies (walrus requires f32r matmul
            # inputs to be produced by a rounding compute op, not a DMA) ----
            aw_sb = consts.tile([128, K, TAPS * C_OUT], cdt, name="aw", tag="aw")
            xt = []
            pooled = consts.tile([128, BPC], f32, name="pooled", tag="pooled")
            with tc.tile_pool(name="scratch", bufs=2) as scratch:
                aw_raw = scratch.tile(
                    [128, K, TAPS * C_OUT], f32, name="awraw", tag="awraw", bufs=1
                )
                nc.sync.dma_start(aw_raw, aw)
                nc.vector.tensor_copy(mm(aw_sb), aw_raw)
                for b in range(BPC):
                    x_raw = scratch.tile([128, HP, WP], f32, name=f"xraw{b}", tag="xraw")
                    nc.sync.dma_start(x_raw, xp[b].rearrange("p (h w) -> p h w", h=HP))
                    x_b = xpool.tile([128, HP, WP], cdt, name=f"x{b}", tag=f"x{b}")
                    nc.vector.tensor_copy(mm(x_b), x_raw)
                    xt.append(x_b)
                    # zero border contributes nothing to the sum
                    nc.vector.reduce_sum(out=pooled[:, b : b + 1], in_=x_b, axis=AX.XY)
            nc.vector.tensor_scalar_mul(pooled, pooled, 1.0 / NPIX)

            # ---- BN1 fold: s1 = g/sqrt(v+eps), t1 = b - m*s1 ----
            eps128 = consts.tile([128, 1], f32, name="eps128", tag="eps128")
            nc.vector.memset(eps128, BN_EPS)
            eps4 = consts.tile([4, 1], f32, name="eps4", tag="eps4")
            nc.vector.memset(eps4, BN_EPS)
            s1 = consts.tile([128, 1], f32, name="s1", tag="s1")
            t1 = consts.tile([128, 1], f32, name="t1", tag="t1")
            nc.scalar.activation(out=s1, in_=bn1_sb[:, 3:4], func=AF.Sqrt, bias=eps128)
            nc.vector.reciprocal(s1, s1)
            nc.vector.tensor_mul(s1, s1, bn1_sb[:, 0:1])
            nc.vector.tensor_mul(t1, bn1_sb[:, 2:3], s1)
            nc.vector.tensor_sub(t1, bn1_sb[:, 1:2], t1)

            with (
                tc.tile_pool(name="wmixp", bufs=1) as wmixp,
                tc.tile_pool(name="outp", bufs=3) as outp,
                tc.tile_pool(name="ps_r", bufs=2, space="PSUM") as psr,
                tc.tile_pool(name="ps_m", bufs=2, space="PSUM") as psm,
                tc.tile_pool(name="ps_c", bufs=1, space="PSUM") as psc,
            ):
                # ---- routing ----
                hp_ = psr.tile([128, BPC], f32, name="hp", tag="r")
                nc.tensor.matmul(hp_, lhsT=wfc1t_sb, rhs=pooled, start=True, stop=True)
                h_sb = consts.tile([128, BPC], f32, name="h", tag="h")
                nc.scalar.activation(out=h_sb, in_=hp_, func=AF.Relu, bias=t1, scale=s1)

                tp = psr.tile([4, K * C_IN], f32, name="tp", tag="r")
                nc.tensor.matmul(tp, lhsT=h_sb, rhs=wfc2t_sb, start=True, stop=True)

                # a2 = s2/T, c2 = ((bfc2 - m2)*s2 + b2)/T   (free-dim vectors, 4 parts)
                a2 = consts.tile([4, 512], f32, name="a2", tag="a2")
                c2 = consts.tile([4, 512], f32, name="c2", tag="c2")
                nc.scalar.activation(out=a2, in_=bn2_sb[:, 3, :], func=AF.Sqrt, bias=eps4)
                nc.vector.reciprocal(a2, a2)
                nc.vector.tensor_mul(a2, a2, bn2_sb[:, 0, :])          # s2
                nc.vector.tensor_sub(c2, bn2_sb[:, 4, :], bn2_sb[:, 2, :])
                nc.vector.tensor_mul(c2, c2, a2)
                nc.vector.tensor_add(c2, c2, bn2_sb[:, 1, :])
                nc.scalar.mul(c2, c2, 1.0 / T_TEMP)
                nc.scalar.mul(a2, a2, 1.0 / T_TEMP)

                z = consts.tile([4, 512], f32, name="z", tag="z")
                nc.vector.tensor_mul(z, tp, a2)
                nc.vector.tensor_add(z, z, c2)

                # softmax rows
                mx = consts.tile([4, 1], f32, name="mx", tag="mx")
                nc.vector.reduce_max(out=mx, in_=z, axis=AX.X)
                negm = consts.tile([4, 1], f32, name="negm", tag="negm")
                nc.vector.tensor_scalar_mul(negm, mx, -1.0)
                e = consts.tile([4, 512], f32, name="e", tag="e")
                nc.scalar.activation(out=e, in_=z, func=AF.Exp, bias=negm, scale=1.0)
                ssum = consts.tile([4, 1], f32, name="ssum", tag="ssum")
                nc.vector.reduce_sum(out=ssum, in_=e, axis=AX.X)
                rs = consts.tile([4, 1], f32, name="rs", tag="rs")
                nc.vector.reciprocal(rs, ssum)
                theta = consts.tile([4, 512], f32, name="theta", tag="theta")
                nc.vector.tensor_scalar_mul(theta, e, rs)

                # thetamean over i within each k block -> [4(b), K]
                tm = consts.tile([4, K], f32, name="tm", tag="tm")
                thv = theta.rearrange("p (k i) -> p k i", k=K)
                for k in range(K):
                    nc.vector.reduce_sum(out=tm[:, k : k + 1], in_=thv[:, k, :], axis=AX.X)
                nc.vector.tensor_scalar_mul(tm, tm, 1.0 / C_IN)

                tmT_p = psr.tile([K, 4], f32, name="tmTp", tag="r")
                nc.tensor.transpose(tmT_p, tm, id4)
                tmT = consts.tile([K, 4], f32, name="tmT", tag="tmT")
                nc.vector.tensor_copy(tmT, tmT_p)

                # bias[o, b] = sum_k ab[k, o] * tmT[k, b]
                bias_sb = consts.tile([128, OC, BPC], f32, name="bias", tag="bias")
                for oc in range(OC):
                    bp = psr.tile([128, BPC], f32, name="bp", tag="r")
                    nc.tensor.matmul(
                        bp, lhsT=ab_sb[:, oc * 128 : (oc + 1) * 128], rhs=tmT,
                        start=True, stop=True,
                    )
                    nc.vector.tensor_copy(bias_sb[:, oc, :], bp)

                # theta transposed to [i, k, b]
                thik = consts.tile([128, K, BPC], f32, name="thik", tag="thik")
                for k in range(K):
                    thp = psr.tile([128, 4], f32, name="thp", tag="r")
                    nc.tensor.transpose(thp, theta[:, k * 128 : (k + 1) * 128], id4)
                    nc.vector.tensor_copy(thik[:, k, :], thp)

                # Dall[i, b, k, :] = id128 * theta[i, b, k]
                Dall = consts.tile([128, BPC, K, 128], cdt, name="dall", tag="dall")
                for b in range(BPC):
                    for k in range(K):
                        nc.vector.tensor_scalar_mul(
                            mm(Dall[:, b, k, :]), id128, thik[:, k, b : b + 1]
                        )

                # ---- weight mixing on PE: wmix[i, b, :] = sum_k Dall[b,k] @ aw[k] ----
                wmix = wmixp.tile([128, BPC, TAPS * C_OUT], cdt, name="wmix", tag="wmix")
                chunks = [(0, 512), (512, 512), (1024, 512), (1536, 512), (2048, 256)]
                for b in range(BPC):
                    for off, ln in chunks:
                        mp = psm.tile([128, 512], f32, name="mix", tag="mix")
                        for k in range(K):
                            nc.tensor.matmul(
                                mp[:, :ln],
                                lhsT=mm(Dall[:, b, k, :]),
                                rhs=mm(aw_sb[:, k, off : off + ln]),
                                start=(k == 0),
                                stop=(k == K - 1),
                            )
                        nc.vector.tensor_copy(
                            mm(wmix[:, b, off : off + ln]), mp[:, :ln]
                        )

                # ---- conv: 9 taps accumulated in psum, tap-outer over groups
                # of 4 pixel-chunk accumulators (4 PSUM banks) for weight
                # reuse; psum->sbuf + bias on the vector engine (ACT is ~7x
                # slower per element on copies) ----
                GRP = 4
                for b in range(BPC):
                    xv = xt[b]
                    for oc in range(OC):
                        for g in range(PCH // GRP):
                            cps = [
                                psc.tile([128, 512], f32, name=f"c{pc}", tag=f"c{pc}")
                                for pc in range(GRP)
                            ]
                            for tap in range(TAPS):
                                ta, tb = divmod(tap, KW)
                                woff = tap * C_OUT + oc * 128
                                lhsT = mm(wmix[:, b, woff : woff + 128])
                                for pc in range(GRP):
                                    r0 = (g * GRP + pc) * ROWS_PER_CHUNK
                                    rhs = xv[:, r0 + ta : r0 + ta + ROWS_PER_CHUNK, tb : tb + W]
                                    nc.tensor.matmul(
                                        cps[pc],
                                        lhsT=lhsT,
                                        rhs=mm(rhs),
                                        start=(tap == 0),
                                        stop=(tap == TAPS - 1),
                                    )
                            ot = outp.tile([128, GRP * 512], f32, name="ot", tag="ot")
                            for pc in range(GRP):
                                nc.vector.tensor_scalar_add(
                                    ot[:, pc * 512 : (pc + 1) * 512],
                                    cps[pc],
                                    bias_sb[:, oc, b : b + 1],
                                )
                            dmae = nc.sync if g % 2 == 0 else nc.scalar
                            dmae.dma_start(out_v[b, oc, g], ot)
    return nc


def _get_nc(split=False):
    with _lock:
        if "nc" not in _cache:
            _cache["nc"] = _build_nc()
        if split and not _cache.get("split_done"):
            # CoreSim can't execute the inserted NoOps, so the wait-split
            # post-pass is applied only when targeting hardware.
            from concourse import mybir

            _split_multi_waits(_cache["nc"], mybir)
            _cache["split_done"] = True
        return _cache["nc"]


def _prep_inputs(inputs):
    """Host-side layout prep (pad / transpose / replicate only)."""
    x = np.asarray(inputs["x"], np.float32)
    xp = np.zeros((B, 128, HP, WP), np.float32)
    xp[:, :, 1 : H + 1, 1 : W + 1] = x
    xp = xp.reshape(B, 128, HP * WP)

    # anchored_w [K, O, I, KH, KW] -> [I, K, KH*KW, O] -> [I, K, 9*O]
    awt = np.ascontiguousarray(
        np.asarray(inputs["anchored_w"], np.float32).transpose(2, 0, 3, 4, 1)
    ).reshape(128, K, TAPS * C_OUT)

    wfc1t = np.ascontiguousarray(np.asarray(inputs["w_fc1"], np.float32).T)
    wfc2t = np.ascontiguousarray(np.asarray(inputs["w_fc2"], np.float32).T)

    bn1 = np.stack(
        [
            np.asarray(inputs["bn1_gamma"], np.float32),
            np.asarray(inputs["bn1_beta"], np.float32),
            np.asarray(inputs["bn1_mean"], np.float32),
            np.asarray(inputs["bn1_var"], np.float32),
        ],
        axis=1,
    )  # [128, 4]

    bn2_rows = np.stack(
        [
            np.asarray(inputs["bn2_gamma"], np.float32),
            np.asarray(inputs["bn2_beta"], np.float32),
            np.asarray(inputs["bn2_mean"], np.float32),
            np.asarray(inputs["bn2_var"], np.float32),
            np.asarray(inputs["b_fc2"], np.float32),
        ],
        axis=0,
    )  # [5, 512]
    bn2 = np.ascontiguousarray(np.broadcast_to(bn2_rows[None], (4, 5, 512)))

    ab = np.ascontiguousarray(np.asarray(inputs["anchored_b"], np.float32))

    shared = {
        "aw": awt,
        "wfc1t": wfc1t,
        "wfc2t": wfc2t,
        "bn1": np.ascontiguousarray(bn1),
        "bn2": bn2,
        "ab": ab,
    }
    in_maps = []
    for c in range(N_CORES):
        m = dict(shared)
        m["xp"] = np.ascontiguousarray(xp[c * BPC : (c + 1) * BPC])
        in_maps.append(m)
    return in_maps


def _get_exec():
    """Build (once) a cached jitted shard_map executable over the 8 cores.

    Mirrors concourse.bass2jax.run_bass_via_pjrt but keeps the jitted
    callable across kernel() invocations so repeat calls skip retracing.
    """
    with _lock:
        if "exec" in _cache:
            return _cache["exec"]

    import jax
    from jax.sharding import Mesh, PartitionSpec
    from jax.experimental.shard_map import shard_map
    from concourse import mybir
    from concourse.bass2jax import (
        _bass_exec_p,
        install_neuronx_cc_hook,
        partition_id_tensor,
    )

    nc = _get_nc(split=True)
    install_neuronx_cc_hook()
    assert nc.dbg_addr is None
    partition_name = nc.partition_id_tensor.name if nc.partition_id_tensor else None

    in_names, out_names, out_avals = [], [], []
    for alloc in nc.m.functions[0].allocations:
        if not isinstance(alloc, mybir.MemoryLocationSet):
            continue
        name = alloc.memorylocations[0].name
        if alloc.kind == "ExternalInput":
            if name != partition_name:
                in_names.append(name)
        elif alloc.kind == "ExternalOutput":
            out_names.append(name)
            out_avals.append(
                jax.core.ShapedArray(tuple(alloc.tensor_shape), mybir.dt.np(alloc.dtype))
            )
    n_params = len(in_names)
    n_outs = len(out_avals)
    all_names = list(in_names) + list(out_names)
    if partition_name is not None:
        all_names.append(partition_name)
    all_names = tuple(all_names)
    donate = tuple(range(n_params, n_params + n_outs))

    def _body(*args):
        operands = list(args)
        if partition_name is not None:
            operands.append(partition_id_tensor())
        outs = _bass_exec_p.bind(
            *operands,
            out_avals=tuple(out_avals),
            in_names=all_names,
            out_names=tuple(out_names),
            lowering_input_output_aliases=(),
            sim_require_finite=True,
            sim_require_nnan=True,
            nc=nc,
        )
        return tuple(outs)

    devices = jax.devices()[:N_CORES]
    mesh = Mesh(np.asarray(devices), ("core",))
    in_specs = (PartitionSpec("core"),) * (n_params + n_outs)
    out_specs = (PartitionSpec("core"),) * n_outs
    sharded = jax.jit(
        shard_map(_body, mesh=mesh, in_specs=in_specs, out_specs=out_specs, check_rep=False),
        donate_argnums=donate,
        keep_unused=True,
    )
    sharding = jax.sharding.NamedSharding(mesh, PartitionSpec("core"))
    zero_shapes = [
        ((N_CORES * a.shape[0],) + tuple(a.shape[1:]), a.dtype) for a in out_avals
    ]
    make_zeros = jax.jit(
        lambda: tuple(np.zeros(s, d) * 0 for s, d in zero_shapes),
        out_shardings=(sharding,) * n_outs,
    )
    ex = {
        "sharded": sharded,
        "in_names": in_names,
        "out_names": out_names,
        "out_avals": out_avals,
        "sharding": sharding,
        "zero_shapes": zero_shapes,
        "make_zeros": make_zeros,
    }
    with _lock:
        _cache["exec"] = ex
    return ex


def _concat_inputs(in_maps, in_names):
    return [
        np.concatenate([in_maps[c][name] for c in range(N_CORES)], axis=0)
        for name in in_names
    ]


def kernel(**inputs):
    ex = _get_exec()
    in_maps = _prep_inputs(inputs)
    concat_in = _concat_inputs(in_maps, ex["in_names"])
    zeros = [np.zeros(s, d) for s, d in ex["zero_shapes"]]
    out_arrs = ex["sharded"](*concat_in, *zeros)
    out = np.asarray(out_arrs[0]).reshape(B, C_OUT, H, W)
    return out


def bench(n_iters=30, **inputs):
    """Steady-state per-iteration device time: inputs resident on device,
    outputs kept on device, async dispatch, block at the end."""
    import time

    import jax
    import jax.numpy as jnp

    ex = _get_exec()
    in_maps = _prep_inputs(inputs)
    concat_in = _concat_inputs(in_maps, ex["in_names"])
    dev_in = [jax.device_put(a, ex["sharding"]) for a in concat_in]
    jax.block_until_ready(dev_in)
    sharded = ex["sharded"]
    sharding = ex["sharding"]
    zero_shapes = ex["zero_shapes"]

    def zeros_on_device():
        return [
            jax.device_put(jnp.zeros(s, d), sharding) for s, d in zero_shapes
        ]

    # warmup
    outs = sharded(*dev_in, *zeros_on_device())
    jax.block_until_ready(outs)

    zsets = [zeros_on_device() for _ in range(n_iters)]
    for z in zsets:
        jax.block_until_ready(z)
    t0 = time.perf_counter()
    last = None
    for i in range(n_iters):
        last = sharded(*dev_in, *zsets[i])
    jax.block_until_ready(last)
    dt = (time.perf_counter() - t0) / n_iters
    return dt * 1e9
